# revision 33
# baseline (speedup 1.0000x reference)
"""Trainium2 Bass kernel for nn_Block_9328668967161.

Computes y = relu(LN_seq(x) @ W1 + b1) @ W2 + b2 + x  where LN_seq
normalizes over the sequence axis (dim 1) with unbiased variance.

Sharding: pure data parallel over the batch axis (32 -> 8 cores x 4).

v3: fp8 DoubleRow matmuls + channel-major bf16 store + epilogue diet.

The engines (not the PE) are the bottleneck once matmuls run at fp8
DoubleRow rate, so the elementwise work is minimized and spread:
  - scales chosen so psum2 == ff exactly (hT = h/16 fp8, W1q = 16*W1,
    aT = relu(h@W1 + b1) true-scale, W2q = W2 unscaled fp8): no rescale
    pass on the mm2 output.
  - b2 is injected INTO the mm2 psum accumulation by a K=1 bf16 matmul
    (lhsT = b2 row, rhs = ones): kills a whole [C,T] elementwise pass.
  - mm1 relu epilogue reads [128,1024] two-bank psum tiles (halves the
    per-instruction overhead), all on ScalarE (activation Relu+bias).
  - residual y = psum2 + xT is ONE tensor_tensor per (kc, jt) on DVE,
    reusing the channel-major bf16 xT that the LN affine needed anyway.
  - LN stats: single bn_stats per [128, T] xT half (batch>0), and the
    mean/var -> scale/shift chain runs packed over [128, KC] once.
  - y stored channel-major bf16; host transposes + upcasts.
Engine budget per batch ~ PE 21us / ScalarE ~22us / DVE ~20us /
GPSIMD (xb casts) ~18us.
"""

import os
import sys

sys.path.insert(0, "/opt/trn_rl_repo")

import numpy as np

import concourse.bass as bass
import concourse.tile as tile
from concourse import bacc
from concourse import mybir
from concourse.bass_utils import run_bass_kernel_spmd
from concourse.masks import make_identity

B, T, C, D = 32, 2048, 256, 1024
N_CORES = 8
BL = B // N_CORES  # batches per core
EPS = 1e-5
KC = C // 128  # 2 channel chunks
KD = D // 128  # 8 dff chunks
NT = T // 128  # 16 token chunks
WS = 16.0  # mm1 weight pre-scale (hT carries 1/WS)

f32 = mybir.dt.float32
bf16 = mybir.dt.bfloat16
fp8 = mybir.dt.float8e4
Alu = mybir.AluOpType
Act = mybir.ActivationFunctionType
DR = mybir.MatmulPerfMode.DoubleRow


def _body(tc, x, gamma, beta, W1, b1, W2, b2, y):
    nc = tc.nc

    from contextlib import ExitStack

    with ExitStack() as ctx:
        consts = ctx.enter_context(tc.tile_pool(name="consts", bufs=1))
        wstage = ctx.enter_context(tc.tile_pool(name="wstage", bufs=1))
        small = ctx.enter_context(tc.tile_pool(name="small", bufs=4))
        xf_pool = ctx.enter_context(tc.tile_pool(name="xf", bufs=3))
        xb_pool = ctx.enter_context(tc.tile_pool(name="xb", bufs=3))
        xT_pool = ctx.enter_context(tc.tile_pool(name="xT", bufs=2))
        hT_pool = ctx.enter_context(tc.tile_pool(name="hT", bufs=2))
        aT_pool = ctx.enter_context(tc.tile_pool(name="aT", bufs=2))
        y_pool = ctx.enter_context(tc.tile_pool(name="ysb", bufs=2))
        psumT = ctx.enter_context(tc.tile_pool(name="psumT", bufs=2, space="PSUM"))
        psum1 = ctx.enter_context(tc.tile_pool(name="psum1", bufs=2, space="PSUM"))
        psum2 = ctx.enter_context(tc.tile_pool(name="psum2", bufs=2, space="PSUM"))

        # ---- constants -------------------------------------------------
        identb = consts.tile([128, 128], bf16)
        make_identity(nc, identb[:])

        # PE warm-up (HAM clock-gate) while batch 0's x DMA lands.
        psw = psum2.tile([128, 512], f32, tag="ps2", name="psw")
        for _ in range(24):
            nc.tensor.matmul(
                psw[:, 0:128], lhsT=identb[:], rhs=identb[:], start=True, stop=True
            )

        # Block token layout: partition p holds tokens [16p, 16p+16); the
        # permutation is self-consistent end-to-end (LN stats permutation-
        # invariant; store is channel-major in the same token order).
        xv = x.rearrange("b (p i) c -> p b i c", i=NT)
        yv = y.rearrange("b (kc p) t -> p b kc t", p=128)

        def load(b):
            xf = xf_pool.tile([128, NT, C], f32, tag="xf", name="xf")
            for g in range(4):
                nc.sync.dma_start(
                    out=xf[:, 4 * g : 4 * g + 4, :], in_=xv[:, b, 4 * g : 4 * g + 4, :]
                )
            return xf

        # Weights: stage fp32 -> fp8. W1q = WS*W1 ; W2q = W2 (unscaled).
        # DMA order = HBM arrival order: w1 (mm1(0) needs it first), x0,
        # w2, x1. The fp8 casts run on DVE (idle early; ScalarE's FIFO
        # must stay clear for batch 0's x casts) -- w2q's cast is emitted
        # AFTER the batch-0 chain so it can't block it.
        xf0 = load(0)

        w1st = wstage.tile([128, KC, D], f32, tag="w1st")
        nc.sync.dma_start(out=w1st[:], in_=W1.rearrange("(kc p) d -> p kc d", p=128))
        w1q = consts.tile([128, KC, D], fp8, tag="w1q")
        nc.vector.tensor_scalar(
            out=w1q[:], in0=w1st[:], scalar1=WS, scalar2=None, op0=Alu.mult
        )

        w2st = wstage.tile([128, KD, C], f32, tag="w2st")
        nc.sync.dma_start(out=w2st[:], in_=W2.rearrange("(kd p) c -> p kd c", p=128))
        w2q = consts.tile([128, KD, C], fp8, tag="w2q")

        xf1 = load(1)

        gam_t = consts.tile([128, KC], f32, tag="gam")
        nc.gpsimd.dma_start(
            out=gam_t[:], in_=gamma.rearrange("(kc p) o -> p (kc o)", p=128)
        )
        bet_t = consts.tile([128, KC], f32, tag="bet")
        nc.gpsimd.dma_start(
            out=bet_t[:], in_=beta.rearrange("(kc p) o -> p (kc o)", p=128)
        )
        # beta/WS: the affine emits hT = h/WS, so shift = beta/WS - mu*scl
        bet_s = consts.tile([128, KC], f32, tag="bets")
        nc.scalar.mul(out=bet_s[:], in_=bet_t[:], mul=1.0 / WS)

        b1t = consts.tile([128, KD], f32, tag="b1t")
        nc.gpsimd.dma_start(out=b1t[:], in_=b1.rearrange("(d p) o -> p (d o)", p=128))
        b1sb = [b1t[:, d : d + 1] for d in range(KD)]

        # b2 as a single-partition bf16 row + a ones row: injected into the
        # mm2 accumulation with a K=1 matmul (out += b2[c] * 1).
        b2st = wstage.tile([1, C], f32, tag="b2st")
        nc.gpsimd.dma_start(out=b2st[:], in_=b2.rearrange("c o -> o c"))
        b2row = consts.tile([1, C], bf16, tag="b2row")
        nc.scalar.copy(out=b2row[:], in_=b2st[:])
        ones_t = consts.tile([1, 512], bf16, tag="ones")
        nc.vector.memset(ones_t[:], 1.0)

        # eps*WS^2 so sqrt((WS^2*T/(T-1))*var + WS^2*eps) = WS*std
        eps_t = consts.tile([128, 1], f32, tag="eps")
        nc.vector.memset(eps_t[:], EPS * WS * WS)

        # ---- per-batch pipeline ---------------------------------------
        def cast(b, xf):
            """fp32 -> bf16 cast for batch b (ScalarE for batch 0's latency,
            GPSIMD otherwise; issued ~2 batches ahead so GPSIMD's slowness
            stays off the critical path)."""
            xb = xb_pool.tile([128, NT, C], bf16, tag="xb", name="xb")
            for g in range(4):
                src = xf[:, 4 * g : 4 * g + 4, :]
                dst = xb[:, 4 * g : 4 * g + 4, :]
                if b == 0:
                    nc.scalar.copy(out=dst, in_=src)
                elif b == 1:
                    # batch 1 fills the pipeline: x1 lands late, so split
                    # the cast across three engines to cut its latency
                    if g < 2:
                        nc.gpsimd.tensor_copy(out=dst, in_=src)
                    elif g == 2:
                        nc.scalar.copy(out=dst, in_=src)
                    else:
                        nc.vector.tensor_copy(out=dst, in_=src)
                else:
                    nc.gpsimd.tensor_copy(out=dst, in_=src)
            return xb

        def prex(b, xb):
            """Transpose + LN stats + affine for batch b. Returns (xT, hT).
            """
            # transpose to channel-major xT[kc] = [128ch, T]; psumT holds 8
            # transposes per [128,1024] tile -> 2 copies per kc.
            xT = [
                xT_pool.tile([128, T], bf16, tag=f"xT{kc}", name=f"xT{kc}")
                for kc in range(KC)
            ]
            stats_t = [
                small.tile([128, 4, 6], f32, tag=f"stats{kc}", name=f"stats{kc}")
                for kc in range(KC)
            ]
            for kc in range(KC):
                for q in range(2):
                    pt = psumT.tile([128, 1024], bf16, tag="psumT", name="pt")
                    for j in range(8):
                        i = q * 8 + j
                        nc.tensor.transpose(
                            out=pt[:, j * 128 : (j + 1) * 128],
                            in_=xb[:, i, kc * 128 : (kc + 1) * 128],
                            identity=identb[:],
                        )
                    with tc.high_priority():
                        nc.vector.tensor_copy(
                            out=xT[kc][:, q * 1024 : (q + 1) * 1024], in_=pt[:]
                        )
                        xTr = xT[kc].rearrange("p (s f) -> p s f", f=512)
                        nc.vector.bn_stats(
                            out=stats_t[kc][:, 2 * q, :], in_=xTr[:, 2 * q, :]
                        )
                        nc.vector.bn_stats(
                            out=stats_t[kc][:, 2 * q + 1, :],
                            in_=xTr[:, 2 * q + 1, :],
                        )

            # LN chain, packed over [128, KC]:
            #   scl = gamma/(WS*std),  shf = beta/WS - mu*scl
            hT = hT_pool.tile([128, KC, T], fp8, tag="hT", name="hT")
            with tc.high_priority():
                mv = small.tile([128, KC, 2], f32, tag="mv", name="mv")
                for kc in range(KC):
                    nc.vector.bn_aggr(out=mv[:, kc, :], in_=stats_t[kc][:])
                stdw = small.tile([128, KC], f32, tag="stdw", name="stdw")
                nc.scalar.activation(
                    out=stdw[:],
                    in_=mv[:, :, 1],
                    func=Act.Sqrt,
                    bias=eps_t[:],
                    scale=WS * WS * float(T) / (T - 1),
                )
                rstw = small.tile([128, KC], f32, tag="rstw", name="rstw")
                nc.vector.reciprocal(out=rstw[:], in_=stdw[:])
                # tiny [128,2] chain: DVE drains between dependent ops make
                # these ~1us each there, so push them to GPSIMD in steady
                # state (batch 0 keeps DVE for latency)
                veng = nc.vector if b == 0 else nc.gpsimd
                scl = small.tile([128, KC], f32, tag="scl", name="scl")
                veng.tensor_mul(out=scl[:], in0=rstw[:], in1=gam_t[:])
                tmp = small.tile([128, KC], f32, tag="tmp", name="tmp")
                veng.tensor_mul(out=tmp[:], in0=mv[:, :, 0], in1=scl[:])
                shf = small.tile([128, KC], f32, tag="shf", name="shf")
                veng.tensor_sub(out=shf[:], in0=bet_s[:], in1=tmp[:])
                for kc in range(KC):
                    if b == 0:
                        # split across engines to cut batch-0 latency
                        nc.scalar.activation(
                            out=hT[:, kc, 0:1024], in_=xT[kc][:, 0:1024],
                            func=Act.Identity,
                            bias=shf[:, kc : kc + 1], scale=scl[:, kc : kc + 1],
                        )
                        nc.vector.tensor_scalar(
                            out=hT[:, kc, 1024:T], in0=xT[kc][:, 1024:T],
                            scalar1=scl[:, kc : kc + 1],
                            scalar2=shf[:, kc : kc + 1],
                            op0=Alu.mult, op1=Alu.add,
                        )
                    elif kc == 0:
                        nc.vector.tensor_scalar(
                            out=hT[:, kc, :], in0=xT[kc][:],
                            scalar1=scl[:, kc : kc + 1],
                            scalar2=shf[:, kc : kc + 1],
                            op0=Alu.mult, op1=Alu.add,
                        )
                    else:
                        # kc=1 half on ScalarE to lighten DVE
                        nc.scalar.activation(
                            out=hT[:, kc, :], in_=xT[kc][:],
                            func=Act.Identity,
                            bias=shf[:, kc : kc + 1], scale=scl[:, kc : kc + 1],
                        )
            return xT, hT

        def mm(b, xT, hT):
            """mm1 + relu + mm2 (+b2) + residual + store for batch b."""
            aT = aT_pool.tile([128, KD, T], fp8, tag="aT", name="aT")
            ysb = y_pool.tile([128, KC, T], bf16, tag="ysb", name="ysb")
            # Both mm1 blocks run before any mm2: mm2(jt) needs all 8 relu
            # tiles of its jtp half, so issuing mm1(jtp=1) in between keeps
            # the PE fed (psum1 rotation paces it) while ScalarE drains the
            # relus -- no multi-us PE gap, HAM stays at 8/8.
            for jtp in range(2):
                for d in range(KD):
                    ps = psum1.tile([128, 1024], f32, tag="psum1", name="ps")
                    for jh in range(2):
                        jt = jtp * 2 + jh
                        nc.tensor.matmul(
                            ps[:, jh * 512 : (jh + 1) * 512],
                            lhsT=w1q[:, 0:KC, d * 128 : (d + 1) * 128],
                            rhs=hT[:, 0:KC, jt * 512 : (jt + 1) * 512],
                            start=True,
                            stop=True,
                            perf_mode=DR,
                        )
                    # relu + b1 -> aT (true scale), one [1024] op; one tile
                    # per batch goes to DVE to balance the engines
                    if jtp == 1 and d == 7:
                        nc.vector.tensor_scalar(
                            out=aT[:, d, jtp * 1024 : (jtp + 1) * 1024],
                            in0=ps[:],
                            scalar1=b1sb[d][:],
                            scalar2=0.0,
                            op0=Alu.add,
                            op1=Alu.max,
                        )
                    else:
                        nc.scalar.activation(
                            out=aT[:, d, jtp * 1024 : (jtp + 1) * 1024],
                            in_=ps[:],
                            func=Act.Relu,
                            bias=b1sb[d][:],
                            scale=1.0,
                        )
            for jt in range(4):
                for kc in range(KC):
                    ps2 = psum2.tile([128, 512], f32, tag="ps2", name="ps2")
                    # b2 first (K=1, bf16): opens the group so the last DR
                    # matmul's stop leads straight into the residual add
                    nc.tensor.matmul(
                        ps2[:],
                        lhsT=b2row[0:1, kc * 128 : (kc + 1) * 128],
                        rhs=ones_t[0:1, :],
                        start=True,
                        stop=False,
                    )
                    for dp in range(KD // 2):
                        nc.tensor.matmul(
                            ps2[:],
                            lhsT=w2q[
                                :, 2 * dp : 2 * dp + 2, kc * 128 : (kc + 1) * 128
                            ],
                            rhs=aT[:, 2 * dp : 2 * dp + 2, jt * 512 : (jt + 1) * 512],
                            start=False,
                            stop=(dp == KD // 2 - 1),
                            perf_mode=DR,
                        )
                    # residual: y = ff + b2 + x  (x via bf16 xT)
                    nc.vector.tensor_add(
                        out=ysb[:, kc, jt * 512 : (jt + 1) * 512],
                        in0=ps2[:],
                        in1=xT[kc][:, jt * 512 : (jt + 1) * 512],
                    )
                if b == BL - 1:
                    # last batch: store per jt column to shorten the tail
                    nc.sync.dma_start(
                        out=yv[:, b, :, jt * 512 : (jt + 1) * 512],
                        in_=ysb[:, :, jt * 512 : (jt + 1) * 512],
                    )
                elif jt == 1 or jt == 3:
                    # store each half as soon as its residuals land
                    jtp = jt // 2
                    nc.sync.dma_start(
                        out=yv[:, b, :, jtp * 1024 : (jtp + 1) * 1024],
                        in_=ysb[:, :, jtp * 1024 : (jtp + 1) * 1024],
                    )

        # software-pipelined emission: loads and casts run TWO batches
        # ahead (GPSIMD cast is slow but off the critical path); the
        # transpose/stats/affine chain runs one batch ahead.
        xb0 = cast(0, xf0)
        xbs = {0: xb0, 1: cast(1, xf1)}
        state = prex(0, xbs.pop(0))
        xT0 = state[0]
        # w2q cast lands on DVE after batch 0's LN chain (w2 arrives ~19us;
        # mm2(0) needs it ~25us)
        nc.vector.tensor_copy(out=w2q[:], in_=w2st[:])
        # filler: keep the PE busy until mm1(0)'s inputs are ready so HAM
        # never re-throttles. The rhs operands REFERENCE batch 0's xb/xT
        # tiles: the dependency keeps the scheduler from hoisting these to
        # the head of the PE stream (a dep-free filler runs before the
        # batch-0 transposes and leaves the same gap it should bridge).
        psw2 = psum2.tile([128, 512], f32, tag="ps2", name="psw2")
        for i in range(48):
            nc.tensor.matmul(
                psw2[:, 0:256],
                lhsT=identb[:],
                rhs=xb0[:, i % NT, 0:256],
                start=True,
                stop=True,
            )
        for i in range(72):
            nc.tensor.matmul(
                psw2[:, 0:256],
                lhsT=identb[:],
                rhs=xT0[i % KC][:, (i % 8) * 256 : (i % 8) * 256 + 256],
                start=True,
                stop=True,
            )
        for b in range(BL):
            # prex(b+1) BEFORE cast(b+2) so GPSIMD's FIFO sees the LN
            # smalls of b+1 ahead of the (slack-rich) next cast
            nxt = prex(b + 1, xbs.pop(b + 1)) if b + 1 < BL else None
            if b + 2 < BL:
                xbs[b + 2] = cast(b + 2, load(b + 2))
            mm(b, *state)
            state = nxt


_CACHED_NC = None


def _build_nc():
    global _CACHED_NC
    if _CACHED_NC is not None:
        return _CACHED_NC
    nc = bacc.Bacc("TRN2", target_bir_lowering=False, debug=False)
    x_d = nc.dram_tensor("x", [BL, T, C], f32, kind="ExternalInput")
    g_d = nc.dram_tensor("gamma", [C, 1], f32, kind="ExternalInput")
    be_d = nc.dram_tensor("beta", [C, 1], f32, kind="ExternalInput")
    w1_d = nc.dram_tensor("W1", [C, D], f32, kind="ExternalInput")
    b1_d = nc.dram_tensor("b1", [D, 1], f32, kind="ExternalInput")
    w2_d = nc.dram_tensor("W2", [D, C], f32, kind="ExternalInput")
    b2_d = nc.dram_tensor("b2", [C, 1], f32, kind="ExternalInput")
    y_d = nc.dram_tensor("y", [BL, C, T], bf16, kind="ExternalOutput")
    with tile.TileContext(nc) as tc:
        _body(
            tc,
            x_d.ap(),
            g_d.ap(),
            be_d.ap(),
            w1_d.ap(),
            b1_d.ap(),
            w2_d.ap(),
            b2_d.ap(),
            y_d.ap(),
        )
    nc.finalize()
    _CACHED_NC = nc
    return nc


def run(inputs, trace=False, **kw):
    nc = _build_nc()
    x = np.ascontiguousarray(np.asarray(inputs["x"], dtype=np.float32))
    gamma = np.asarray(inputs["gamma"], dtype=np.float32).reshape(C, 1)
    beta = np.asarray(inputs["beta"], dtype=np.float32).reshape(C, 1)
    W1 = np.ascontiguousarray(np.asarray(inputs["W1"], dtype=np.float32))
    b1 = np.asarray(inputs["b1"], dtype=np.float32).reshape(D, 1)
    W2 = np.ascontiguousarray(np.asarray(inputs["W2"], dtype=np.float32))
    b2 = np.asarray(inputs["b2"], dtype=np.float32).reshape(C, 1)

    in_maps = []
    for c in range(N_CORES):
        in_maps.append(
            {
                "x": x[c * BL : (c + 1) * BL],
                "gamma": gamma,
                "beta": beta,
                "W1": W1,
                "b1": b1,
                "W2": W2,
                "b2": b2,
            }
        )
    res = run_bass_kernel_spmd(nc, in_maps, list(range(N_CORES)), trace=trace, **kw)
    # y comes back channel-major [BL, C, T'] bf16 with the block-token
    # permutation on T': free position i*128 + p  <->  token p*16 + i.
    ys = []
    for c in range(N_CORES):
        ycm = np.asarray(res.results[c]["y"]).astype(np.float32)  # [BL, C, T']
        ytc = ycm.transpose(0, 2, 1)  # [BL, T', C]
        ytc = ytc.reshape(BL, NT, 128, C).transpose(0, 2, 1, 3).reshape(BL, T, C)
        ys.append(ytc)
    y = np.concatenate(ys, axis=0)
    return y, res


def kernel(**inputs):
    y, _ = run(inputs, trace=False)
    return y


# revision 34
# speedup vs baseline: 1.0465x; 1.0465x over previous
"""Trainium2 Bass kernel for nn_Block_9328668967161.

Computes y = relu(LN_seq(x) @ W1 + b1) @ W2 + b2 + x  where LN_seq
normalizes over the sequence axis (dim 1) with unbiased variance.

Sharding: pure data parallel over the batch axis (32 -> 8 cores x 4).

v3: fp8 DoubleRow matmuls + channel-major bf16 store + epilogue diet.

The engines (not the PE) are the bottleneck once matmuls run at fp8
DoubleRow rate, so the elementwise work is minimized and spread:
  - scales chosen so psum2 == ff exactly (hT = h/16 fp8, W1q = 16*W1,
    aT = relu(h@W1 + b1) true-scale, W2q = W2 unscaled fp8): no rescale
    pass on the mm2 output.
  - b2 is injected INTO the mm2 psum accumulation by a K=1 bf16 matmul
    (lhsT = b2 row, rhs = ones): kills a whole [C,T] elementwise pass.
  - mm1 relu epilogue reads [128,1024] two-bank psum tiles (halves the
    per-instruction overhead), all on ScalarE (activation Relu+bias).
  - residual y = psum2 + xT is ONE tensor_tensor per (kc, jt) on DVE,
    reusing the channel-major bf16 xT that the LN affine needed anyway.
  - LN stats: single bn_stats per [128, T] xT half (batch>0), and the
    mean/var -> scale/shift chain runs packed over [128, KC] once.
  - y stored channel-major bf16; host transposes + upcasts.
Engine budget per batch ~ PE 21us / ScalarE ~22us / DVE ~20us /
GPSIMD (xb casts) ~18us.
"""

import os
import sys

sys.path.insert(0, "/opt/trn_rl_repo")

import numpy as np

import concourse.bass as bass
import concourse.tile as tile
from concourse import bacc
from concourse import mybir
from concourse.bass_utils import run_bass_kernel_spmd
from concourse.masks import make_identity

B, T, C, D = 32, 2048, 256, 1024
N_CORES = 8
BL = B // N_CORES  # batches per core
EPS = 1e-5
KC = C // 128  # 2 channel chunks
KD = D // 128  # 8 dff chunks
NT = T // 128  # 16 token chunks
WS = 16.0  # mm1 weight pre-scale (hT carries 1/WS)

f32 = mybir.dt.float32
bf16 = mybir.dt.bfloat16
fp8 = mybir.dt.float8e4
Alu = mybir.AluOpType
Act = mybir.ActivationFunctionType
DR = mybir.MatmulPerfMode.DoubleRow


def _body(tc, x, gamma, beta, W1, b1, W2, b2, y):
    nc = tc.nc

    from contextlib import ExitStack

    with ExitStack() as ctx:
        consts = ctx.enter_context(tc.tile_pool(name="consts", bufs=1))
        wstage = ctx.enter_context(tc.tile_pool(name="wstage", bufs=1))
        small = ctx.enter_context(tc.tile_pool(name="small", bufs=4))
        xf_pool = ctx.enter_context(tc.tile_pool(name="xf", bufs=3))
        xb_pool = ctx.enter_context(tc.tile_pool(name="xb", bufs=3))
        xT_pool = ctx.enter_context(tc.tile_pool(name="xT", bufs=2))
        hT_pool = ctx.enter_context(tc.tile_pool(name="hT", bufs=2))
        aT_pool = ctx.enter_context(tc.tile_pool(name="aT", bufs=2))
        y_pool = ctx.enter_context(tc.tile_pool(name="ysb", bufs=2))
        psumT = ctx.enter_context(tc.tile_pool(name="psumT", bufs=2, space="PSUM"))
        psum1 = ctx.enter_context(tc.tile_pool(name="psum1", bufs=2, space="PSUM"))
        psum2 = ctx.enter_context(tc.tile_pool(name="psum2", bufs=2, space="PSUM"))

        # ---- constants -------------------------------------------------
        identb = consts.tile([128, 128], bf16)
        make_identity(nc, identb[:])

        # PE warm-up (HAM clock-gate) while batch 0's x DMA lands.
        psw = psum2.tile([128, 512], f32, tag="ps2", name="psw")
        for _ in range(24):
            nc.tensor.matmul(
                psw[:, 0:128], lhsT=identb[:], rhs=identb[:], start=True, stop=True
            )

        # Block token layout: partition p holds tokens [16p, 16p+16); the
        # permutation is self-consistent end-to-end (LN stats permutation-
        # invariant; store is channel-major in the same token order).
        xv = x.rearrange("b (p i) c -> p b i c", i=NT)
        yv = y.rearrange("b (kc p) t -> p b kc t", p=128)

        def load(b):
            xf = xf_pool.tile([128, NT, C], f32, tag="xf", name="xf")
            for g in range(4):
                nc.sync.dma_start(
                    out=xf[:, 4 * g : 4 * g + 4, :], in_=xv[:, b, 4 * g : 4 * g + 4, :]
                )
            return xf

        # Weights: stage fp32 -> fp8. W1q = WS*W1 ; W2q = W2 (unscaled).
        # DMA order = HBM arrival order: w1 (mm1(0) needs it first), x0,
        # w2, x1. The fp8 casts run on DVE (idle early; ScalarE's FIFO
        # must stay clear for batch 0's x casts) -- w2q's cast is emitted
        # AFTER the batch-0 chain so it can't block it.
        xf0 = load(0)

        w1st = wstage.tile([128, KC, D], f32, tag="w1st")
        nc.sync.dma_start(out=w1st[:], in_=W1.rearrange("(kc p) d -> p kc d", p=128))
        w1q = consts.tile([128, KC, D], fp8, tag="w1q")
        nc.vector.tensor_scalar(
            out=w1q[:], in0=w1st[:], scalar1=WS, scalar2=None, op0=Alu.mult
        )

        w2st = wstage.tile([128, KD, C], f32, tag="w2st")
        nc.sync.dma_start(out=w2st[:], in_=W2.rearrange("(kd p) c -> p kd c", p=128))
        w2q = consts.tile([128, KD, C], fp8, tag="w2q")

        xf1 = load(1)

        gam_t = consts.tile([128, KC], f32, tag="gam")
        nc.gpsimd.dma_start(
            out=gam_t[:], in_=gamma.rearrange("(kc p) o -> p (kc o)", p=128)
        )
        bet_t = consts.tile([128, KC], f32, tag="bet")
        nc.gpsimd.dma_start(
            out=bet_t[:], in_=beta.rearrange("(kc p) o -> p (kc o)", p=128)
        )
        # beta/WS: the affine emits hT = h/WS, so shift = beta/WS - mu*scl
        bet_s = consts.tile([128, KC], f32, tag="bets")
        nc.scalar.mul(out=bet_s[:], in_=bet_t[:], mul=1.0 / WS)

        b1t = consts.tile([128, KD], f32, tag="b1t")
        nc.gpsimd.dma_start(out=b1t[:], in_=b1.rearrange("(d p) o -> p (d o)", p=128))
        b1sb = [b1t[:, d : d + 1] for d in range(KD)]

        # b2 as a single-partition bf16 row + a ones row: injected into the
        # mm2 accumulation with a K=1 matmul (out += b2[c] * 1).
        b2st = wstage.tile([1, C], f32, tag="b2st")
        nc.gpsimd.dma_start(out=b2st[:], in_=b2.rearrange("c o -> o c"))
        b2row = consts.tile([1, C], bf16, tag="b2row")
        nc.scalar.copy(out=b2row[:], in_=b2st[:])
        ones_t = consts.tile([1, 512], bf16, tag="ones")
        nc.vector.memset(ones_t[:], 1.0)

        # eps*WS^2 so sqrt((WS^2*T/(T-1))*var + WS^2*eps) = WS*std
        eps_t = consts.tile([128, 1], f32, tag="eps")
        nc.vector.memset(eps_t[:], EPS * WS * WS)

        # ---- per-batch pipeline ---------------------------------------
        def cast(b, xf):
            """fp32 -> bf16 cast for batch b (ScalarE for batch 0's latency,
            GPSIMD otherwise; issued ~2 batches ahead so GPSIMD's slowness
            stays off the critical path)."""
            xb = xb_pool.tile([128, NT, C], bf16, tag="xb", name="xb")
            for g in range(4):
                src = xf[:, 4 * g : 4 * g + 4, :]
                dst = xb[:, 4 * g : 4 * g + 4, :]
                if b == 0:
                    nc.scalar.copy(out=dst, in_=src)
                elif b == 1:
                    # batch 1 fills the pipeline: x1 lands late, so split
                    # the cast across three engines to cut its latency
                    if g < 2:
                        nc.gpsimd.tensor_copy(out=dst, in_=src)
                    elif g == 2:
                        nc.scalar.copy(out=dst, in_=src)
                    else:
                        nc.vector.tensor_copy(out=dst, in_=src)
                else:
                    nc.gpsimd.tensor_copy(out=dst, in_=src)
            return xb

        def prex(b, xb):
            """Transpose + LN stats + affine for batch b. Returns (xT, hT).
            """
            # transpose to channel-major xT[kc] = [128ch, T]; psumT holds 8
            # transposes per [128,1024] tile -> 2 copies per kc.
            xT = [
                xT_pool.tile([128, T], bf16, tag=f"xT{kc}", name=f"xT{kc}")
                for kc in range(KC)
            ]
            stats_t = [
                small.tile([128, 4, 6], f32, tag=f"stats{kc}", name=f"stats{kc}")
                for kc in range(KC)
            ]
            for kc in range(KC):
                for q in range(2):
                    pt = psumT.tile([128, 1024], bf16, tag="psumT", name="pt")
                    for j in range(8):
                        i = q * 8 + j
                        nc.tensor.transpose(
                            out=pt[:, j * 128 : (j + 1) * 128],
                            in_=xb[:, i, kc * 128 : (kc + 1) * 128],
                            identity=identb[:],
                        )
                    with tc.high_priority():
                        nc.vector.tensor_copy(
                            out=xT[kc][:, q * 1024 : (q + 1) * 1024], in_=pt[:]
                        )
                        xTr = xT[kc].rearrange("p (s f) -> p s f", f=512)
                        nc.vector.bn_stats(
                            out=stats_t[kc][:, 2 * q, :], in_=xTr[:, 2 * q, :]
                        )
                        nc.vector.bn_stats(
                            out=stats_t[kc][:, 2 * q + 1, :],
                            in_=xTr[:, 2 * q + 1, :],
                        )

            # LN chain, packed over [128, KC]:
            #   scl = gamma/(WS*std),  shf = beta/WS - mu*scl
            hT = hT_pool.tile([128, KC, T], fp8, tag="hT", name="hT")
            with tc.high_priority():
                mv = small.tile([128, KC, 2], f32, tag="mv", name="mv")
                for kc in range(KC):
                    nc.vector.bn_aggr(out=mv[:, kc, :], in_=stats_t[kc][:])
                stdw = small.tile([128, KC], f32, tag="stdw", name="stdw")
                nc.scalar.activation(
                    out=stdw[:],
                    in_=mv[:, :, 1],
                    func=Act.Sqrt,
                    bias=eps_t[:],
                    scale=WS * WS * float(T) / (T - 1),
                )
                rstw = small.tile([128, KC], f32, tag="rstw", name="rstw")
                nc.vector.reciprocal(out=rstw[:], in_=stdw[:])
                scl = small.tile([128, KC], f32, tag="scl", name="scl")
                nc.vector.tensor_mul(out=scl[:], in0=rstw[:], in1=gam_t[:])
                tmp = small.tile([128, KC], f32, tag="tmp", name="tmp")
                nc.vector.tensor_mul(out=tmp[:], in0=mv[:, :, 0], in1=scl[:])
                shf = small.tile([128, KC], f32, tag="shf", name="shf")
                nc.vector.tensor_sub(out=shf[:], in0=bet_s[:], in1=tmp[:])
                for kc in range(KC):
                    if b == 0:
                        # split across engines to cut batch-0 latency
                        nc.scalar.activation(
                            out=hT[:, kc, 0:1024], in_=xT[kc][:, 0:1024],
                            func=Act.Identity,
                            bias=shf[:, kc : kc + 1], scale=scl[:, kc : kc + 1],
                        )
                        nc.vector.tensor_scalar(
                            out=hT[:, kc, 1024:T], in0=xT[kc][:, 1024:T],
                            scalar1=scl[:, kc : kc + 1],
                            scalar2=shf[:, kc : kc + 1],
                            op0=Alu.mult, op1=Alu.add,
                        )
                    else:
                        nc.vector.tensor_scalar(
                            out=hT[:, kc, :], in0=xT[kc][:],
                            scalar1=scl[:, kc : kc + 1],
                            scalar2=shf[:, kc : kc + 1],
                            op0=Alu.mult, op1=Alu.add,
                        )
            return xT, hT

        def mm(b, xT, hT):
            """mm1 + relu + mm2 (+b2) + residual + store for batch b."""
            aT = aT_pool.tile([128, KD, T], fp8, tag="aT", name="aT")
            ysb = y_pool.tile([128, KC, T], bf16, tag="ysb", name="ysb")
            # Both mm1 blocks run before any mm2: mm2(jt) needs all 8 relu
            # tiles of its jtp half, so issuing mm1(jtp=1) in between keeps
            # the PE fed (psum1 rotation paces it) while ScalarE drains the
            # relus -- no multi-us PE gap, HAM stays at 8/8.
            for jtp in range(2):
                for d in range(KD):
                    ps = psum1.tile([128, 1024], f32, tag="psum1", name="ps")
                    for jh in range(2):
                        jt = jtp * 2 + jh
                        nc.tensor.matmul(
                            ps[:, jh * 512 : (jh + 1) * 512],
                            lhsT=w1q[:, 0:KC, d * 128 : (d + 1) * 128],
                            rhs=hT[:, 0:KC, jt * 512 : (jt + 1) * 512],
                            start=True,
                            stop=True,
                            perf_mode=DR,
                        )
                    # relu + b1 -> aT (true scale), one [1024] op; one tile
                    # per batch goes to DVE to balance the engines
                    if jtp == 1 and d == 7:
                        nc.vector.tensor_scalar(
                            out=aT[:, d, jtp * 1024 : (jtp + 1) * 1024],
                            in0=ps[:],
                            scalar1=b1sb[d][:],
                            scalar2=0.0,
                            op0=Alu.add,
                            op1=Alu.max,
                        )
                    else:
                        nc.scalar.activation(
                            out=aT[:, d, jtp * 1024 : (jtp + 1) * 1024],
                            in_=ps[:],
                            func=Act.Relu,
                            bias=b1sb[d][:],
                            scale=1.0,
                        )
            for jt in range(4):
                for kc in range(KC):
                    ps2 = psum2.tile([128, 512], f32, tag="ps2", name="ps2")
                    # b2 first (K=1, bf16): opens the group so the last DR
                    # matmul's stop leads straight into the residual add
                    nc.tensor.matmul(
                        ps2[:],
                        lhsT=b2row[0:1, kc * 128 : (kc + 1) * 128],
                        rhs=ones_t[0:1, :],
                        start=True,
                        stop=False,
                    )
                    for dp in range(KD // 2):
                        nc.tensor.matmul(
                            ps2[:],
                            lhsT=w2q[
                                :, 2 * dp : 2 * dp + 2, kc * 128 : (kc + 1) * 128
                            ],
                            rhs=aT[:, 2 * dp : 2 * dp + 2, jt * 512 : (jt + 1) * 512],
                            start=False,
                            stop=(dp == KD // 2 - 1),
                            perf_mode=DR,
                        )
                    # residual: y = ff + b2 + x  (x via bf16 xT)
                    nc.vector.tensor_add(
                        out=ysb[:, kc, jt * 512 : (jt + 1) * 512],
                        in0=ps2[:],
                        in1=xT[kc][:, jt * 512 : (jt + 1) * 512],
                    )
                if b == BL - 1:
                    # last batch: store per jt column to shorten the tail
                    nc.sync.dma_start(
                        out=yv[:, b, :, jt * 512 : (jt + 1) * 512],
                        in_=ysb[:, :, jt * 512 : (jt + 1) * 512],
                    )
                elif jt == 1 or jt == 3:
                    # store each half as soon as its residuals land
                    jtp = jt // 2
                    nc.sync.dma_start(
                        out=yv[:, b, :, jtp * 1024 : (jtp + 1) * 1024],
                        in_=ysb[:, :, jtp * 1024 : (jtp + 1) * 1024],
                    )

        # software-pipelined emission: loads and casts run TWO batches
        # ahead (GPSIMD cast is slow but off the critical path); the
        # transpose/stats/affine chain runs one batch ahead.
        xb0 = cast(0, xf0)
        xbs = {0: xb0, 1: cast(1, xf1)}
        state = prex(0, xbs.pop(0))
        xT0 = state[0]
        # w2q cast lands on DVE after batch 0's LN chain (w2 arrives ~19us;
        # mm2(0) needs it ~25us)
        nc.vector.tensor_copy(out=w2q[:], in_=w2st[:])
        # filler: keep the PE busy until mm1(0)'s inputs are ready so HAM
        # never re-throttles. The rhs operands REFERENCE batch 0's xb/xT
        # tiles: the dependency keeps the scheduler from hoisting these to
        # the head of the PE stream (a dep-free filler runs before the
        # batch-0 transposes and leaves the same gap it should bridge).
        psw2 = psum2.tile([128, 512], f32, tag="ps2", name="psw2")
        for i in range(48):
            nc.tensor.matmul(
                psw2[:, 0:256],
                lhsT=identb[:],
                rhs=xb0[:, i % NT, 0:256],
                start=True,
                stop=True,
            )
        for i in range(72):
            nc.tensor.matmul(
                psw2[:, 0:256],
                lhsT=identb[:],
                rhs=xT0[i % KC][:, (i % 8) * 256 : (i % 8) * 256 + 256],
                start=True,
                stop=True,
            )
        for b in range(BL):
            # prex(b+1) BEFORE cast(b+2) so GPSIMD's FIFO sees the LN
            # smalls of b+1 ahead of the (slack-rich) next cast
            nxt = prex(b + 1, xbs.pop(b + 1)) if b + 1 < BL else None
            if b + 2 < BL:
                xbs[b + 2] = cast(b + 2, load(b + 2))
            mm(b, *state)
            state = nxt


_CACHED_NC = None


def _build_nc():
    global _CACHED_NC
    if _CACHED_NC is not None:
        return _CACHED_NC
    nc = bacc.Bacc("TRN2", target_bir_lowering=False, debug=False)
    x_d = nc.dram_tensor("x", [BL, T, C], f32, kind="ExternalInput")
    g_d = nc.dram_tensor("gamma", [C, 1], f32, kind="ExternalInput")
    be_d = nc.dram_tensor("beta", [C, 1], f32, kind="ExternalInput")
    w1_d = nc.dram_tensor("W1", [C, D], f32, kind="ExternalInput")
    b1_d = nc.dram_tensor("b1", [D, 1], f32, kind="ExternalInput")
    w2_d = nc.dram_tensor("W2", [D, C], f32, kind="ExternalInput")
    b2_d = nc.dram_tensor("b2", [C, 1], f32, kind="ExternalInput")
    y_d = nc.dram_tensor("y", [BL, C, T], bf16, kind="ExternalOutput")
    with tile.TileContext(nc) as tc:
        _body(
            tc,
            x_d.ap(),
            g_d.ap(),
            be_d.ap(),
            w1_d.ap(),
            b1_d.ap(),
            w2_d.ap(),
            b2_d.ap(),
            y_d.ap(),
        )
    nc.finalize()
    _CACHED_NC = nc
    return nc


def run(inputs, trace=False, **kw):
    nc = _build_nc()
    x = np.ascontiguousarray(np.asarray(inputs["x"], dtype=np.float32))
    gamma = np.asarray(inputs["gamma"], dtype=np.float32).reshape(C, 1)
    beta = np.asarray(inputs["beta"], dtype=np.float32).reshape(C, 1)
    W1 = np.ascontiguousarray(np.asarray(inputs["W1"], dtype=np.float32))
    b1 = np.asarray(inputs["b1"], dtype=np.float32).reshape(D, 1)
    W2 = np.ascontiguousarray(np.asarray(inputs["W2"], dtype=np.float32))
    b2 = np.asarray(inputs["b2"], dtype=np.float32).reshape(C, 1)

    in_maps = []
    for c in range(N_CORES):
        in_maps.append(
            {
                "x": x[c * BL : (c + 1) * BL],
                "gamma": gamma,
                "beta": beta,
                "W1": W1,
                "b1": b1,
                "W2": W2,
                "b2": b2,
            }
        )
    res = run_bass_kernel_spmd(nc, in_maps, list(range(N_CORES)), trace=trace, **kw)
    # y comes back channel-major [BL, C, T'] bf16 with the block-token
    # permutation on T': free position i*128 + p  <->  token p*16 + i.
    ys = []
    for c in range(N_CORES):
        ycm = np.asarray(res.results[c]["y"]).astype(np.float32)  # [BL, C, T']
        ytc = ycm.transpose(0, 2, 1)  # [BL, T', C]
        ytc = ytc.reshape(BL, NT, 128, C).transpose(0, 2, 1, 3).reshape(BL, T, C)
        ys.append(ytc)
    y = np.concatenate(ys, axis=0)
    return y, res


def kernel(**inputs):
    y, _ = run(inputs, trace=False)
    return y
